# revision 3
# baseline (speedup 1.0000x reference)
"""MoE (top-2 of 6 experts, dense-expert reference semantics) on 8 TRN2 cores.

Strategy: data-parallel over tokens (8192 tokens -> 1024/core), experts
replicated. Per core:
  - gating in fp32 on the tensor engine (top-2 margins are ~1e-5, bf16 would
    flip selections), top-2 + softmax via vector/scalar engines,
  - per-expert MLP in bf16 (x^T layout, [feature, token]): h^T = W1^T @ x^T,
    gelu(+b1) on ACT, combine-weight fold into h^T on DVE, then the second
    matmul accumulates all experts' contributions plus the combine@b2 term.
  - output written [D, T] per core; host transposes and concatenates.
No collectives needed.
"""

import sys

sys.path.insert(0, "/opt/trn_rl_repo")

import numpy as np
import ml_dtypes

import concourse.bass as bass  # noqa: F401  (registers engine classes)
import concourse.bacc as bacc
import concourse.mybir as mybir
from concourse import tile
from concourse import bass_utils

AF = mybir.ActivationFunctionType
ALU = mybir.AluOpType
AX = mybir.AxisListType
BF16 = mybir.dt.bfloat16
F32 = mybir.dt.float32

N_CORES = 8
B, S, D, E, H = 4, 2048, 1024, 6, 2048
TOKENS = B * S
T = TOKENS // N_CORES  # 1024 tokens per core
TC = 512               # token chunk = matmul moving free dim
NCH = T // TC          # 2 chunks
DB = D // 128          # 8 d blocks
JB = H // 128          # 16 hidden blocks
TB = T // 128          # 8 token blocks (gating)
NEG_BIG = -1.0e30


def _build_program():
    nc = bacc.Bacc("TRN2", target_bir_lowering=False, debug=False,
                   num_devices=N_CORES)

    xt_f = nc.dram_tensor("xt_f", [D, T], F32, kind="ExternalInput").ap()
    xt_b = nc.dram_tensor("xt_b", [D, T], BF16, kind="ExternalInput").ap()
    w1 = nc.dram_tensor("w1", [E, D, H], BF16, kind="ExternalInput").ap()
    w2 = nc.dram_tensor("w2", [E, H, D], BF16, kind="ExternalInput").ap()
    wg = nc.dram_tensor("wg", [D, E], F32, kind="ExternalInput").ap()
    bg = nc.dram_tensor("bg", [1, E], F32, kind="ExternalInput").ap()
    b1r = nc.dram_tensor("b1r", [128, E * JB], F32, kind="ExternalInput").ap()
    b2 = nc.dram_tensor("b2", [E, D], BF16, kind="ExternalInput").ap()
    ones = nc.dram_tensor("ones", [1, 128], F32, kind="ExternalInput").ap()
    sel = nc.dram_tensor("sel", [E, E * 128], BF16, kind="ExternalInput").ap()
    eye = nc.dram_tensor("eye", [128, 128], F32, kind="ExternalInput").ap()
    out = nc.dram_tensor("out", [D, T], F32, kind="ExternalOutput").ap()

    with tile.TileContext(nc) as tc:
        with (
            tc.tile_pool(name="constp", bufs=1) as constp,
            tc.tile_pool(name="xtfp", bufs=4) as xtfp,
            tc.tile_pool(name="xtbp", bufs=DB) as xtbp,
            tc.tile_pool(name="w1p", bufs=10) as w1p,
            tc.tile_pool(name="w2p", bufs=18) as w2p,
            tc.tile_pool(name="htp", bufs=18) as htp,
            tc.tile_pool(name="yaccp", bufs=DB) as yaccp,
            tc.tile_pool(name="crepp", bufs=E * NCH) as crepp,
            tc.tile_pool(name="gatp", bufs=2) as gatp,
            tc.tile_pool(name="psA", bufs=3, space="PSUM") as psA,
            tc.tile_pool(name="psB", bufs=3, space="PSUM") as psB,
        ):
            # ---- constants ----
            eye_sb = constp.tile([128, 128], F32, name="eye_sb", tag="eye")
            nc.sync.dma_start(eye_sb[:], eye[:])
            ones_sb = constp.tile([1, 128], F32, name="ones_sb", tag="ones")
            nc.sync.dma_start(ones_sb[:], ones[:])
            bg_sb = constp.tile([1, E], F32, name="bg_sb", tag="bg")
            nc.sync.dma_start(bg_sb[:], bg[:])
            sel_sb = constp.tile([E, E * 128], BF16, name="sel_sb", tag="sel")
            nc.sync.dma_start(sel_sb[:], sel[:])
            b1_sb = constp.tile([128, E * JB], F32, name="b1_sb", tag="b1")
            nc.sync.dma_start(b1_sb[:], b1r[:])
            b2_sb = constp.tile([E, D], BF16, name="b2_sb", tag="b2")
            nc.sync.dma_start(b2_sb[:], b2[:])
            wg_sb = []
            for d in range(DB):
                wgt = constp.tile([128, E], F32, name=f"wg_sb{d}", tag=f"wg{d}")
                nc.sync.dma_start(wgt[:], wg[d * 128:(d + 1) * 128, :])
                wg_sb.append(wgt)
            combT = constp.tile([E, T], BF16, name="combT", tag="combT")

            # ---- resident bf16 x^T ----
            xtb = []
            for d in range(DB):
                xt = xtbp.tile([128, T], BF16, name=f"xtb{d}", tag="xtb")
                nc.sync.dma_start(xt[:], xt_b[d * 128:(d + 1) * 128, :])
                xtb.append(xt)

            # ---- prime expert-0 weights so mm1 can start as soon as possible
            w1t0 = []
            for d in range(DB):
                wt = w1p.tile([128, H], BF16, name=f"w1t0_{d}", tag="w1")
                nc.sync.dma_start(wt[:], w1[0, d * 128:(d + 1) * 128, :])
                w1t0.append(wt)
            w2t0 = []
            for j in range(JB):
                wt = w2p.tile([128, D], BF16, name=f"w2t0_{j}", tag="w2")
                nc.sync.dma_start(wt[:], w2[0, j * 128:(j + 1) * 128, :])
                w2t0.append(wt)

            # ---- gating (fp32), phase-batched so the top-2 chains pipeline
            lgs, cmbs = [], []
            for tb in range(TB):
                ps_g = psA.tile([128, E], F32, name="ps_g", tag="psA")
                for d in range(DB):
                    xg = xtfp.tile([128, 128], F32, name="xg", tag="xg")
                    nc.sync.dma_start(
                        xg[:], xt_f[d * 128:(d + 1) * 128,
                                    tb * 128:(tb + 1) * 128])
                    nc.tensor.matmul(ps_g[:], xg[:], wg_sb[d][:],
                                     start=(d == 0), stop=False)
                nc.tensor.matmul(ps_g[:], ones_sb[:], bg_sb[:],
                                 start=False, stop=True)
                lg = gatp.tile([128, E], F32, name=f"lg{tb}", tag=f"lg{tb}")
                nc.vector.tensor_copy(lg[:], ps_g[:])
                lgs.append(lg)
            for tb in range(TB):
                lg = lgs[tb]
                m1 = gatp.tile([128, 1], F32, name="m1", tag="m1")
                nc.vector.reduce_max(m1[:], lg[:], axis=AX.X)
                eq1 = gatp.tile([128, E], F32, name="eq1", tag="eq1")
                nc.vector.tensor_scalar(eq1[:], lg[:], m1[:], None,
                                        ALU.is_equal)
                mk = gatp.tile([128, E], F32, name="mk", tag="mk")
                nc.vector.scalar_tensor_tensor(mk[:], eq1[:], NEG_BIG, lg[:],
                                               ALU.mult, ALU.add)
                m2 = gatp.tile([128, 1], F32, name="m2", tag="m2")
                nc.vector.reduce_max(m2[:], mk[:], axis=AX.X)
                eq2 = gatp.tile([128, E], F32, name="eq2", tag="eq2")
                nc.vector.tensor_scalar(eq2[:], mk[:], m2[:], None,
                                        ALU.is_equal)
                dd = gatp.tile([128, 1], F32, name="dd", tag="dd")
                nc.vector.tensor_sub(dd[:], m2[:], m1[:])
                w2s = gatp.tile([128, 1], F32, name="w2s", tag="w2s")
                nc.scalar.activation(w2s[:], dd[:], AF.Sigmoid)
                w1s = gatp.tile([128, 1], F32, name="w1s", tag="w1s")
                nc.vector.tensor_scalar(w1s[:], w2s[:], -1.0, 1.0,
                                        ALU.mult, ALU.add)
                cb1 = gatp.tile([128, E], F32, name="cb1", tag="cb1")
                nc.vector.tensor_scalar(cb1[:], eq1[:], w1s[:], None, ALU.mult)
                cmb = gatp.tile([128, E], F32, name=f"cmb{tb}", tag=f"cmb{tb}")
                nc.vector.scalar_tensor_tensor(cmb[:], eq2[:], w2s[:], cb1[:],
                                               ALU.mult, ALU.add)
                cmbs.append(cmb)
            for tb in range(TB):
                ps_t = psA.tile([E, 128], F32, name="ps_t", tag="psA")
                nc.tensor.transpose(ps_t[:], cmbs[tb][:], eye_sb[:])
                nc.vector.tensor_copy(combT[:, tb * 128:(tb + 1) * 128],
                                      ps_t[:])

            # ---- combine weights broadcast across partitions (bf16) ----
            crep = [[None] * NCH for _ in range(E)]
            for e in range(E):
                for c in range(NCH):
                    ps_c = psA.tile([128, TC], F32, name="ps_c", tag="psA")
                    nc.tensor.matmul(ps_c[:],
                                     sel_sb[:, e * 128:(e + 1) * 128],
                                     combT[:, c * TC:(c + 1) * TC],
                                     start=True, stop=True)
                    cr = crepp.tile([128, TC], BF16, name=f"crep{e}_{c}",
                                    tag="crep")
                    nc.vector.tensor_copy(cr[:], ps_c[:])
                    crep[e][c] = cr

            # ---- expert loop ----
            yacc = []
            for d in range(DB):
                ya = yaccp.tile([128, T], F32, name=f"yacc{d}", tag="yacc")
                yacc.append(ya)

            for e in range(E):
                if e == 0:
                    w1t, w2t = w1t0, w2t0
                else:
                    w1t = []
                    for d in range(DB):
                        wt = w1p.tile([128, H], BF16, name=f"w1t{e}_{d}",
                                      tag="w1")
                        nc.sync.dma_start(wt[:],
                                          w1[e, d * 128:(d + 1) * 128, :])
                        w1t.append(wt)
                    w2t = []
                    for j in range(JB):
                        wt = w2p.tile([128, D], BF16, name=f"w2t{e}_{j}",
                                      tag="w2")
                        nc.sync.dma_start(wt[:],
                                          w2[e, j * 128:(j + 1) * 128, :])
                        w2t.append(wt)

                for c in range(NCH):
                    ht_list = []
                    for j in range(JB):
                        ps1 = psA.tile([128, TC], F32, name="ps1", tag="psA")
                        for d in range(DB):
                            nc.tensor.matmul(
                                ps1[:],
                                w1t[d][:, j * 128:(j + 1) * 128],
                                xtb[d][:, c * TC:(c + 1) * TC],
                                start=(d == 0), stop=(d == DB - 1))
                        ht = htp.tile([128, TC], BF16, name="ht", tag="ht")
                        nc.scalar.activation(
                            ht[:], ps1[:], AF.Gelu,
                            bias=b1_sb[:, e * JB + j:e * JB + j + 1])
                        nc.vector.tensor_tensor(ht[:], ht[:], crep[e][c][:],
                                                ALU.mult)
                        ht_list.append(ht)
                    for d2 in range(DB):
                        ps2 = psB.tile([128, TC], F32, name="ps2", tag="psB")
                        if e == 0:
                            nc.tensor.matmul(
                                ps2[:], b2_sb[:, d2 * 128:(d2 + 1) * 128],
                                combT[:, c * TC:(c + 1) * TC],
                                start=True, stop=False)
                        for j2 in range(JB):
                            nc.tensor.matmul(
                                ps2[:],
                                w2t[j2][:, d2 * 128:(d2 + 1) * 128],
                                ht_list[j2][:],
                                start=(e != 0 and j2 == 0),
                                stop=(j2 == JB - 1))
                        ysl = yacc[d2][:, c * TC:(c + 1) * TC]
                        if e == 0:
                            nc.vector.tensor_copy(ysl, ps2[:])
                        else:
                            nc.vector.tensor_tensor(ysl, ysl, ps2[:], ALU.add)

            for d in range(DB):
                nc.sync.dma_start(out[d * 128:(d + 1) * 128, :], yacc[d][:])

    nc.compile()
    return nc


_PROG = None


def _get_program():
    global _PROG
    if _PROG is None:
        _PROG = _build_program()
    return _PROG


def kernel(x, Wg, bg, W1, b1, W2, b2):
    nc = _get_program()

    xf = np.ascontiguousarray(x.reshape(TOKENS, D).astype(np.float32))
    W1b = np.ascontiguousarray(W1.astype(ml_dtypes.bfloat16))
    W2b = np.ascontiguousarray(W2.astype(ml_dtypes.bfloat16))
    b2b = np.ascontiguousarray(b2.astype(ml_dtypes.bfloat16))
    b1r = np.ascontiguousarray(
        b1.reshape(E, JB, 128).transpose(2, 0, 1).reshape(128, E * JB)
    ).astype(np.float32)
    ones_f = np.ones((1, 128), np.float32)
    eye_f = np.eye(128, dtype=np.float32)
    sel_b = np.zeros((E, E * 128), ml_dtypes.bfloat16)
    for e in range(E):
        sel_b[e, e * 128:(e + 1) * 128] = 1.0

    in_maps = []
    for c in range(N_CORES):
        xt = np.ascontiguousarray(xf[c * T:(c + 1) * T].T)  # [D, T] fp32
        in_maps.append({
            "xt_f": xt,
            "xt_b": xt.astype(ml_dtypes.bfloat16),
            "w1": W1b,
            "w2": W2b,
            "wg": np.ascontiguousarray(Wg.astype(np.float32)),
            "bg": np.ascontiguousarray(bg.astype(np.float32)).reshape(1, E),
            "b1r": b1r,
            "b2": b2b,
            "ones": ones_f,
            "sel": sel_b,
            "eye": eye_f,
        })

    res = bass_utils.run_bass_kernel_spmd(nc, in_maps,
                                          core_ids=list(range(N_CORES)))
    parts = [res.results[c]["out"].T for c in range(N_CORES)]  # [T, D] each
    return np.concatenate(parts, axis=0).reshape(B, S, D).astype(np.float32)


# revision 18
# speedup vs baseline: 1.0405x; 1.0405x over previous
"""MoE (top-2 of 6 experts, dense-expert reference semantics) on 8 TRN2 cores.

Strategy: data-parallel over tokens (8192 tokens -> 1024/core), experts
replicated. Per core:
  - gating in fp32 on the tensor engine (top-2 margins are ~1e-5, bf16 would
    flip selections), top-2 + softmax via vector/scalar engines,
  - per-expert MLP in bf16 (x^T layout, [feature, token]): h^T = W1^T @ x^T,
    gelu(+b1) on ACT, combine-weight fold into h^T on DVE, then the second
    matmul accumulates all experts' contributions plus the combine@b2 term.
  - output written [D, T] per core; host transposes and concatenates.
No collectives needed.
"""

import sys

sys.path.insert(0, "/opt/trn_rl_repo")

import numpy as np
import ml_dtypes

import concourse.bass as bass  # noqa: F401  (registers engine classes)
import concourse.bacc as bacc
import concourse.mybir as mybir
from concourse import tile
from concourse import bass_utils

AF = mybir.ActivationFunctionType
ALU = mybir.AluOpType
AX = mybir.AxisListType
BF16 = mybir.dt.bfloat16
F32 = mybir.dt.float32

N_CORES = 8
B, S, D, E, H = 4, 2048, 1024, 6, 2048
TOKENS = B * S
T = TOKENS // N_CORES  # 1024 tokens per core
TC = 512               # token chunk = matmul moving free dim
NCH = T // TC          # 2 chunks
DB = D // 128          # 8 d blocks
JB = H // 128          # 16 hidden blocks
TB = T // 128          # 8 token blocks (gating)
NEG_BIG = -1.0e30


def _build_program():
    nc = bacc.Bacc("TRN2", target_bir_lowering=False, debug=False,
                   num_devices=N_CORES)

    xt_f = nc.dram_tensor("xt_f", [D, T], F32, kind="ExternalInput").ap()
    xt_b = nc.dram_tensor("xt_b", [D, T], BF16, kind="ExternalInput").ap()
    w1 = nc.dram_tensor("w1", [E, D, H], BF16, kind="ExternalInput").ap()
    w2 = nc.dram_tensor("w2", [E, H, D], BF16, kind="ExternalInput").ap()
    wg = nc.dram_tensor("wg", [D, E], F32, kind="ExternalInput").ap()
    bg = nc.dram_tensor("bg", [1, E], F32, kind="ExternalInput").ap()
    b1r = nc.dram_tensor("b1r", [128, E * JB], F32, kind="ExternalInput").ap()
    b2 = nc.dram_tensor("b2", [E, D], BF16, kind="ExternalInput").ap()
    ones = nc.dram_tensor("ones", [1, 128], F32, kind="ExternalInput").ap()
    sel = nc.dram_tensor("sel", [E, E * 128], BF16, kind="ExternalInput").ap()
    eye = nc.dram_tensor("eye", [128, 128], F32, kind="ExternalInput").ap()
    out = nc.dram_tensor("out", [D, T], F32, kind="ExternalOutput").ap()

    with tile.TileContext(nc) as tc:
        with (
            tc.tile_pool(name="constp", bufs=1) as constp,
            tc.tile_pool(name="xtfp", bufs=16) as xtfp,
            tc.tile_pool(name="xtbp", bufs=2 * DB) as xtbp,
            tc.tile_pool(name="w1p", bufs=20) as w1p,
            tc.tile_pool(name="w2p", bufs=18) as w2p,
            tc.tile_pool(name="htp", bufs=2 * JB + 2) as htp,
            tc.tile_pool(name="yaccp", bufs=DB) as yaccp,
            tc.tile_pool(name="crepp", bufs=E * NCH) as crepp,
            tc.tile_pool(name="gatp", bufs=4) as gatp,
            tc.tile_pool(name="psA", bufs=4, space="PSUM") as psA,
            tc.tile_pool(name="psB", bufs=4, space="PSUM") as psB,
        ):
            # ---- constants ----
            eye_sb = constp.tile([128, 128], F32, name="eye_sb", tag="eye")
            nc.sync.dma_start(eye_sb[:], eye[:])
            ones_sb = constp.tile([1, 128], F32, name="ones_sb", tag="ones")
            nc.sync.dma_start(ones_sb[:], ones[:])
            bg_sb = constp.tile([1, E], F32, name="bg_sb", tag="bg")
            nc.sync.dma_start(bg_sb[:], bg[:])
            sel_sb = constp.tile([E, E * 128], BF16, name="sel_sb", tag="sel")
            nc.sync.dma_start(sel_sb[:], sel[:])
            b1_sb = constp.tile([128, E * JB], F32, name="b1_sb", tag="b1")
            nc.sync.dma_start(b1_sb[:], b1r[:])
            b2_sb = constp.tile([E, D], BF16, name="b2_sb", tag="b2")
            nc.sync.dma_start(b2_sb[:], b2[:])
            wg_sb = []
            for d in range(DB):
                wgt = constp.tile([128, E], F32, name=f"wg_sb{d}", tag=f"wg{d}")
                nc.sync.dma_start(wgt[:], wg[d * 128:(d + 1) * 128, :])
                wg_sb.append(wgt)
            combT = constp.tile([E, T], BF16, name="combT", tag="combT")

            # ---- priming order: gating x tiles stream first (tiny, lets PE
            # start ~3us in), then expert-0 W1 first half + chunk-0 x^T so
            # mm1 can follow right after gating; the rest stream underneath.
            HH = H // 2
            w1t0 = [[None, None] for _ in range(DB)]
            xtb = [[None, None] for _ in range(DB)]

            # ---- gating (fp32), phase-batched so the top-2 chains pipeline
            lgs, cmbs = [], []
            for tb in range(TB):
                ps_g = psB.tile([128, E], F32, name="ps_g", tag="psB")
                for d in range(DB):
                    xg = xtfp.tile([128, 128], F32, name="xg", tag="xg")
                    nc.sync.dma_start(
                        xg[:], xt_f[d * 128:(d + 1) * 128,
                                    tb * 128:(tb + 1) * 128])
                    nc.tensor.matmul(ps_g[:], xg[:], wg_sb[d][:],
                                     start=(d == 0), stop=False)
                nc.tensor.matmul(ps_g[:], ones_sb[:], bg_sb[:],
                                 start=False, stop=True)
                lg = gatp.tile([128, E], F32, name=f"lg{tb}", tag=f"lg{tb}")
                nc.vector.tensor_copy(lg[:], ps_g[:])
                lgs.append(lg)

            # deferred priming: expert-0 weights + x^T (mm1 inputs first)
            for d in range(DB):
                wa = w1p.tile([128, HH], BF16, name=f"w1t0_{d}a", tag="w1")
                nc.sync.dma_start(wa[:], w1[0, d * 128:(d + 1) * 128, 0:HH])
                w1t0[d][0] = wa
            for d in range(DB):
                xa = xtbp.tile([128, TC], BF16, name=f"xtb{d}a", tag="xtb")
                nc.sync.dma_start(xa[:], xt_b[d * 128:(d + 1) * 128, 0:TC])
                xtb[d][0] = xa
            for d in range(DB):
                wb = w1p.tile([128, HH], BF16, name=f"w1t0_{d}b", tag="w1")
                nc.sync.dma_start(wb[:], w1[0, d * 128:(d + 1) * 128, HH:H])
                w1t0[d][1] = wb
            for d in range(DB):
                xb = xtbp.tile([128, TC], BF16, name=f"xtb{d}b", tag="xtb")
                nc.sync.dma_start(xb[:], xt_b[d * 128:(d + 1) * 128, TC:T])
                xtb[d][1] = xb
            w2t0 = []
            for j in range(JB):
                wt = w2p.tile([128, D], BF16, name=f"w2t0_{j}", tag="w2")
                nc.sync.dma_start(wt[:], w2[0, j * 128:(j + 1) * 128, :])
                w2t0.append(wt)

            def topk_chain(tb):
                lg = lgs[tb]
                m1 = gatp.tile([128, 1], F32, name="m1", tag="m1")
                nc.vector.reduce_max(m1[:], lg[:], axis=AX.X)
                eq1 = gatp.tile([128, E], F32, name="eq1", tag="eq1")
                nc.vector.tensor_scalar(eq1[:], lg[:], m1[:], None,
                                        ALU.is_equal)
                mk = gatp.tile([128, E], F32, name="mk", tag="mk")
                nc.vector.scalar_tensor_tensor(mk[:], eq1[:], NEG_BIG, lg[:],
                                               ALU.mult, ALU.add)
                m2 = gatp.tile([128, 1], F32, name="m2", tag="m2")
                nc.vector.reduce_max(m2[:], mk[:], axis=AX.X)
                eq2 = gatp.tile([128, E], F32, name="eq2", tag="eq2")
                nc.vector.tensor_scalar(eq2[:], mk[:], m2[:], None,
                                        ALU.is_equal)
                dd = gatp.tile([128, 1], F32, name="dd", tag="dd")
                nc.vector.tensor_sub(dd[:], m2[:], m1[:])
                w2s = gatp.tile([128, 1], F32, name="w2s", tag="w2s")
                nc.scalar.activation(w2s[:], dd[:], AF.Sigmoid)
                w1s = gatp.tile([128, 1], F32, name="w1s", tag="w1s")
                nc.vector.tensor_scalar(w1s[:], w2s[:], -1.0, 1.0,
                                        ALU.mult, ALU.add)
                cb1 = gatp.tile([128, E], F32, name="cb1", tag="cb1")
                nc.vector.tensor_scalar(cb1[:], eq1[:], w1s[:], None, ALU.mult)
                cmb = gatp.tile([128, E], F32, name=f"cmb{tb}", tag=f"cmb{tb}")
                nc.vector.scalar_tensor_tensor(cmb[:], eq2[:], w2s[:], cb1[:],
                                               ALU.mult, ALU.add)
                cmbs.append(cmb)

            # process gating tail in chunk-halves so chunk-0 combine
            # weights (and crep tiles) are ready as early as possible
            crep = [[None] * NCH for _ in range(E)]
            TBH = TB // NCH
            for half in range(NCH):
                for tb in range(half * TBH, (half + 1) * TBH):
                    topk_chain(tb)
                for tb in range(half * TBH, (half + 1) * TBH):
                    ps_t = psB.tile([E, 128], F32, name="ps_t", tag="psB")
                    nc.tensor.transpose(ps_t[:], cmbs[tb][:], eye_sb[:])
                    nc.vector.tensor_copy(combT[:, tb * 128:(tb + 1) * 128],
                                          ps_t[:])
                c = half
                for e in range(E):
                    ps_c = psB.tile([128, TC], F32, name="ps_c", tag="psB")
                    nc.tensor.matmul(ps_c[:],
                                     sel_sb[:, e * 128:(e + 1) * 128],
                                     combT[:, c * TC:(c + 1) * TC],
                                     start=True, stop=True)
                    cr = crepp.tile([128, TC], BF16, name=f"crep{e}_{c}",
                                    tag="crep")
                    nc.vector.tensor_copy(cr[:], ps_c[:])
                    crep[e][c] = cr

            # ---- expert loop ----
            yacc = []
            for d in range(DB):
                ya = yaccp.tile([128, T], F32, name=f"yacc{d}", tag="yacc")
                yacc.append(ya)

            for e in range(E):
                if e == 0:
                    w1t, w2t = w1t0, w2t0
                else:
                    w1t = []
                    for d in range(DB):
                        halves = []
                        for h2 in range(2):
                            wt = w1p.tile([128, HH], BF16,
                                          name=f"w1t{e}_{d}{'ab'[h2]}",
                                          tag="w1")
                            nc.sync.dma_start(
                                wt[:], w1[e, d * 128:(d + 1) * 128,
                                          h2 * HH:(h2 + 1) * HH])
                            halves.append(wt)
                        w1t.append(halves)
                    w2t = []
                    for j in range(JB):
                        wt = w2p.tile([128, D], BF16, name=f"w2t{e}_{j}",
                                      tag="w2")
                        nc.sync.dma_start(wt[:],
                                          w2[e, j * 128:(j + 1) * 128, :])
                        w2t.append(wt)

                # mm1: chunk-paired so each stationary W1 slice loads once
                ht_list = [[None] * JB for _ in range(NCH)]
                for j in range(JB):
                    ps1c = [psA.tile([128, TC], F32, name=f"ps1_{c}",
                                     tag="psA") for c in range(NCH)]
                    h2, jj = divmod(j, JB // 2)
                    for d in range(DB):
                        for c in range(NCH):
                            nc.tensor.matmul(
                                ps1c[c][:],
                                w1t[d][h2][:, jj * 128:(jj + 1) * 128],
                                xtb[d][c][:],
                                start=(d == 0), stop=(d == DB - 1))
                    for c in range(NCH):
                        ht = htp.tile([128, TC], BF16, name=f"ht_{c}",
                                      tag="ht")
                        nc.scalar.activation(
                            ht[:], ps1c[c][:], AF.Gelu,
                            bias=b1_sb[:, e * JB + j:e * JB + j + 1])
                        nc.vector.tensor_tensor(ht[:], ht[:], crep[e][c][:],
                                                ALU.mult)
                        ht_list[c][j] = ht
                # mm2: chunk-paired, accumulate into yacc; on the last
                # expert stream each finished d-block straight to DRAM
                for d2 in range(DB):
                    ps2c = [psB.tile([128, TC], F32, name=f"ps2_{c}",
                                     tag="psB") for c in range(NCH)]
                    if e == 0:
                        for c in range(NCH):
                            nc.tensor.matmul(
                                ps2c[c][:], b2_sb[:, d2 * 128:(d2 + 1) * 128],
                                combT[:, c * TC:(c + 1) * TC],
                                start=True, stop=False)
                    for j2 in range(JB):
                        for c in range(NCH):
                            nc.tensor.matmul(
                                ps2c[c][:],
                                w2t[j2][:, d2 * 128:(d2 + 1) * 128],
                                ht_list[c][j2][:],
                                start=(e != 0 and j2 == 0),
                                stop=(j2 == JB - 1))
                    for c in range(NCH):
                        ysl = yacc[d2][:, c * TC:(c + 1) * TC]
                        if e == 0:
                            nc.vector.tensor_copy(ysl, ps2c[c][:])
                        else:
                            nc.vector.tensor_tensor(ysl, ysl, ps2c[c][:],
                                                    ALU.add)
                    if e == E - 1:
                        nc.sync.dma_start(out[d2 * 128:(d2 + 1) * 128, :],
                                          yacc[d2][:])

    nc.compile()
    return nc


_PROG = None


def _get_program():
    global _PROG
    if _PROG is None:
        _PROG = _build_program()
    return _PROG


def kernel(x, Wg, bg, W1, b1, W2, b2):
    nc = _get_program()

    xf = np.ascontiguousarray(x.reshape(TOKENS, D).astype(np.float32))
    W1b = np.ascontiguousarray(W1.astype(ml_dtypes.bfloat16))
    W2b = np.ascontiguousarray(W2.astype(ml_dtypes.bfloat16))
    b2b = np.ascontiguousarray(b2.astype(ml_dtypes.bfloat16))
    b1r = np.ascontiguousarray(
        b1.reshape(E, JB, 128).transpose(2, 0, 1).reshape(128, E * JB)
    ).astype(np.float32)
    ones_f = np.ones((1, 128), np.float32)
    eye_f = np.eye(128, dtype=np.float32)
    sel_b = np.zeros((E, E * 128), ml_dtypes.bfloat16)
    for e in range(E):
        sel_b[e, e * 128:(e + 1) * 128] = 1.0

    in_maps = []
    for c in range(N_CORES):
        xt = np.ascontiguousarray(xf[c * T:(c + 1) * T].T)  # [D, T] fp32
        in_maps.append({
            "xt_f": xt,
            "xt_b": xt.astype(ml_dtypes.bfloat16),
            "w1": W1b,
            "w2": W2b,
            "wg": np.ascontiguousarray(Wg.astype(np.float32)),
            "bg": np.ascontiguousarray(bg.astype(np.float32)).reshape(1, E),
            "b1r": b1r,
            "b2": b2b,
            "ones": ones_f,
            "sel": sel_b,
            "eye": eye_f,
        })

    res = bass_utils.run_bass_kernel_spmd(nc, in_maps,
                                          core_ids=list(range(N_CORES)))
    parts = [res.results[c]["out"].T for c in range(N_CORES)]  # [T, D] each
    return np.concatenate(parts, axis=0).reshape(B, S, D).astype(np.float32)


# revision 19
# speedup vs baseline: 1.2393x; 1.1910x over previous
"""MoE (top-2 of 6 experts, dense-expert reference semantics) on 8 TRN2 cores.

Strategy: data-parallel over tokens (8192 tokens -> 1024/core), experts
replicated. Per core:
  - gating in fp32 on the tensor engine (top-2 margins are ~1e-5, bf16 would
    flip selections), top-2 + softmax via vector/scalar engines,
  - per-expert MLP in bf16 (x^T layout, [feature, token]): h^T = W1^T @ x^T,
    gelu(+b1) on ACT, combine-weight fold into h^T on DVE, then the second
    matmul accumulates all experts' contributions plus the combine@b2 term.
  - output written [D, T] per core; host transposes and concatenates.
No collectives needed.
"""

import sys

sys.path.insert(0, "/opt/trn_rl_repo")

import numpy as np
import ml_dtypes

import concourse.bass as bass  # noqa: F401  (registers engine classes)
import concourse.bacc as bacc
import concourse.mybir as mybir
from concourse import tile
from concourse import bass_utils

AF = mybir.ActivationFunctionType
ALU = mybir.AluOpType
AX = mybir.AxisListType
BF16 = mybir.dt.bfloat16
F32 = mybir.dt.float32

N_CORES = 8
B, S, D, E, H = 4, 2048, 1024, 6, 2048
TOKENS = B * S
T = TOKENS // N_CORES  # 1024 tokens per core
TC = 512               # token chunk = matmul moving free dim
NCH = T // TC          # 2 chunks
DB = D // 128          # 8 d blocks
JB = H // 128          # 16 hidden blocks
TB = T // 128          # 8 token blocks (gating)
NEG_BIG = -1.0e30


def _build_program():
    nc = bacc.Bacc("TRN2", target_bir_lowering=False, debug=False,
                   num_devices=N_CORES)

    xt_f = nc.dram_tensor("xt_f", [D, T], F32, kind="ExternalInput").ap()
    xt_b = nc.dram_tensor("xt_b", [D, T], BF16, kind="ExternalInput").ap()
    w1 = nc.dram_tensor("w1", [E, D, H], BF16, kind="ExternalInput").ap()
    w2 = nc.dram_tensor("w2", [E, H, D], BF16, kind="ExternalInput").ap()
    wg = nc.dram_tensor("wg", [D, E], F32, kind="ExternalInput").ap()
    bg = nc.dram_tensor("bg", [1, E], F32, kind="ExternalInput").ap()
    b1r = nc.dram_tensor("b1r", [128, E * JB], F32, kind="ExternalInput").ap()
    b2 = nc.dram_tensor("b2", [E, D], BF16, kind="ExternalInput").ap()
    ones = nc.dram_tensor("ones", [1, 128], F32, kind="ExternalInput").ap()
    sel = nc.dram_tensor("sel", [E, E * 128], BF16, kind="ExternalInput").ap()
    eye = nc.dram_tensor("eye", [128, 128], F32, kind="ExternalInput").ap()
    out = nc.dram_tensor("out", [D, T], F32, kind="ExternalOutput").ap()

    with tile.TileContext(nc) as tc:
        with (
            tc.tile_pool(name="constp", bufs=1) as constp,
            tc.tile_pool(name="xtfp", bufs=16) as xtfp,
            tc.tile_pool(name="xtbp", bufs=2 * DB) as xtbp,
            tc.tile_pool(name="w1p", bufs=20) as w1p,
            tc.tile_pool(name="w2p", bufs=18) as w2p,
            tc.tile_pool(name="htp", bufs=2 * JB + 2) as htp,
            tc.tile_pool(name="yaccp", bufs=DB) as yaccp,
            tc.tile_pool(name="crepp", bufs=E * NCH) as crepp,
            tc.tile_pool(name="gatp", bufs=4) as gatp,
            tc.tile_pool(name="psA", bufs=4, space="PSUM") as psA,
            tc.tile_pool(name="psB", bufs=4, space="PSUM") as psB,
        ):
            # ---- constants ----
            eye_sb = constp.tile([128, 128], F32, name="eye_sb", tag="eye")
            nc.sync.dma_start(eye_sb[:], eye[:])
            ones_sb = constp.tile([1, 128], F32, name="ones_sb", tag="ones")
            nc.sync.dma_start(ones_sb[:], ones[:])
            bg_sb = constp.tile([1, E], F32, name="bg_sb", tag="bg")
            nc.sync.dma_start(bg_sb[:], bg[:])
            sel_sb = constp.tile([E, E * 128], BF16, name="sel_sb", tag="sel")
            nc.sync.dma_start(sel_sb[:], sel[:])
            b1_sb = constp.tile([128, E * JB], F32, name="b1_sb", tag="b1")
            nc.sync.dma_start(b1_sb[:], b1r[:])
            b2_sb = constp.tile([E, D], BF16, name="b2_sb", tag="b2")
            nc.sync.dma_start(b2_sb[:], b2[:])
            wg_sb = []
            for d in range(DB):
                wgt = constp.tile([128, E], F32, name=f"wg_sb{d}", tag=f"wg{d}")
                nc.sync.dma_start(wgt[:], wg[d * 128:(d + 1) * 128, :])
                wg_sb.append(wgt)
            combT = constp.tile([E, T], BF16, name="combT", tag="combT")

            # ---- priming order: gating x tiles stream first (tiny, lets PE
            # start ~3us in), then expert-0 W1 first half + chunk-0 x^T so
            # mm1 can follow right after gating; the rest stream underneath.
            HH = H // 2
            w1t0 = [[None, None] for _ in range(DB)]
            xtb = [[None, None] for _ in range(DB)]

            # ---- gating (fp32), phase-batched so the top-2 chains pipeline
            lgs, cmbs = [], []
            for tb in range(TB):
                ps_g = psB.tile([128, E], F32, name="ps_g", tag="psB")
                for d in range(DB):
                    xg = xtfp.tile([128, 128], F32, name="xg", tag="xg")
                    nc.sync.dma_start(
                        xg[:], xt_f[d * 128:(d + 1) * 128,
                                    tb * 128:(tb + 1) * 128])
                    nc.tensor.matmul(ps_g[:], xg[:], wg_sb[d][:],
                                     start=(d == 0), stop=False)
                nc.tensor.matmul(ps_g[:], ones_sb[:], bg_sb[:],
                                 start=False, stop=True)
                lg = gatp.tile([128, E], F32, name=f"lg{tb}", tag=f"lg{tb}")
                nc.vector.tensor_copy(lg[:], ps_g[:])
                lgs.append(lg)

            # deferred priming: expert-0 weights + x^T (mm1 inputs first)
            for d in range(DB):
                wa = w1p.tile([128, HH], BF16, name=f"w1t0_{d}a", tag="w1")
                nc.sync.dma_start(wa[:], w1[0, d * 128:(d + 1) * 128, 0:HH])
                w1t0[d][0] = wa
            for d in range(DB):
                xa = xtbp.tile([128, TC], BF16, name=f"xtb{d}a", tag="xtb")
                nc.sync.dma_start(xa[:], xt_b[d * 128:(d + 1) * 128, 0:TC])
                xtb[d][0] = xa
            for d in range(DB):
                wb = w1p.tile([128, HH], BF16, name=f"w1t0_{d}b", tag="w1")
                nc.sync.dma_start(wb[:], w1[0, d * 128:(d + 1) * 128, HH:H])
                w1t0[d][1] = wb
            for d in range(DB):
                xb = xtbp.tile([128, TC], BF16, name=f"xtb{d}b", tag="xtb")
                nc.sync.dma_start(xb[:], xt_b[d * 128:(d + 1) * 128, TC:T])
                xtb[d][1] = xb
            w2t0 = []
            for j in range(JB):
                wt = w2p.tile([128, D], BF16, name=f"w2t0_{j}", tag="w2")
                nc.sync.dma_start(wt[:], w2[0, j * 128:(j + 1) * 128, :])
                w2t0.append(wt)

            def topk_chain(tb):
                lg = lgs[tb]
                m1 = gatp.tile([128, 1], F32, name="m1", tag="m1")
                nc.vector.reduce_max(m1[:], lg[:], axis=AX.X)
                eq1 = gatp.tile([128, E], F32, name="eq1", tag="eq1")
                nc.vector.tensor_scalar(eq1[:], lg[:], m1[:], None,
                                        ALU.is_equal)
                mk = gatp.tile([128, E], F32, name="mk", tag="mk")
                nc.vector.scalar_tensor_tensor(mk[:], eq1[:], NEG_BIG, lg[:],
                                               ALU.mult, ALU.add)
                m2 = gatp.tile([128, 1], F32, name="m2", tag="m2")
                nc.vector.reduce_max(m2[:], mk[:], axis=AX.X)
                eq2 = gatp.tile([128, E], F32, name="eq2", tag="eq2")
                nc.vector.tensor_scalar(eq2[:], mk[:], m2[:], None,
                                        ALU.is_equal)
                dd = gatp.tile([128, 1], F32, name="dd", tag="dd")
                nc.vector.tensor_sub(dd[:], m2[:], m1[:])
                w2s = gatp.tile([128, 1], F32, name="w2s", tag="w2s")
                nc.scalar.activation(w2s[:], dd[:], AF.Sigmoid)
                w1s = gatp.tile([128, 1], F32, name="w1s", tag="w1s")
                nc.vector.tensor_scalar(w1s[:], w2s[:], -1.0, 1.0,
                                        ALU.mult, ALU.add)
                cb1 = gatp.tile([128, E], F32, name="cb1", tag="cb1")
                nc.vector.tensor_scalar(cb1[:], eq1[:], w1s[:], None, ALU.mult)
                cmb = gatp.tile([128, E], F32, name=f"cmb{tb}", tag=f"cmb{tb}")
                nc.vector.scalar_tensor_tensor(cmb[:], eq2[:], w2s[:], cb1[:],
                                               ALU.mult, ALU.add)
                cmbs.append(cmb)

            # process gating tail in chunk-halves so chunk-0 combine
            # weights (and crep tiles) are ready as early as possible
            crep = [[None] * NCH for _ in range(E)]
            TBH = TB // NCH
            for half in range(NCH):
                for tb in range(half * TBH, (half + 1) * TBH):
                    topk_chain(tb)
                for tb in range(half * TBH, (half + 1) * TBH):
                    ps_t = psB.tile([E, 128], F32, name="ps_t", tag="psB")
                    nc.tensor.transpose(ps_t[:], cmbs[tb][:], eye_sb[:])
                    nc.vector.tensor_copy(combT[:, tb * 128:(tb + 1) * 128],
                                          ps_t[:])
                c = half
                for e in range(E):
                    ps_c = psB.tile([128, TC], F32, name="ps_c", tag="psB")
                    nc.tensor.matmul(ps_c[:],
                                     sel_sb[:, e * 128:(e + 1) * 128],
                                     combT[:, c * TC:(c + 1) * TC],
                                     start=True, stop=True)
                    cr = crepp.tile([128, TC], BF16, name=f"crep{e}_{c}",
                                    tag="crep")
                    nc.vector.tensor_copy(cr[:], ps_c[:])
                    crep[e][c] = cr

            # ---- expert loop ----
            yacc = []
            for d in range(DB):
                ya = yaccp.tile([128, T], F32, name=f"yacc{d}", tag="yacc")
                yacc.append(ya)

            for e in range(E):
                if e == 0:
                    w1t, w2t = w1t0, w2t0
                else:
                    w1t = []
                    for d in range(DB):
                        halves = []
                        for h2 in range(2):
                            wt = w1p.tile([128, HH], BF16,
                                          name=f"w1t{e}_{d}{'ab'[h2]}",
                                          tag="w1")
                            nc.sync.dma_start(
                                wt[:], w1[e, d * 128:(d + 1) * 128,
                                          h2 * HH:(h2 + 1) * HH])
                            halves.append(wt)
                        w1t.append(halves)
                    w2t = []
                    for j in range(JB):
                        wt = w2p.tile([128, D], BF16, name=f"w2t{e}_{j}",
                                      tag="w2")
                        nc.sync.dma_start(wt[:],
                                          w2[e, j * 128:(j + 1) * 128, :])
                        w2t.append(wt)

                # mm1: chunk-paired so each stationary W1 slice loads once
                ht_list = [[None] * JB for _ in range(NCH)]
                for j in range(JB):
                    ps1c = [psA.tile([128, TC], F32, name=f"ps1_{c}",
                                     tag="psA") for c in range(NCH)]
                    h2, jj = divmod(j, JB // 2)
                    for d in range(DB):
                        for c in range(NCH):
                            nc.tensor.matmul(
                                ps1c[c][:],
                                w1t[d][h2][:, jj * 128:(jj + 1) * 128],
                                xtb[d][c][:],
                                start=(d == 0), stop=(d == DB - 1))
                    for c in range(NCH):
                        ht = htp.tile([128, TC], BF16, name=f"ht_{c}",
                                      tag="ht")
                        nc.scalar.activation(
                            ht[:], ps1c[c][:], AF.Gelu,
                            bias=b1_sb[:, e * JB + j:e * JB + j + 1])
                        nc.vector.tensor_tensor(ht[:], ht[:], crep[e][c][:],
                                                ALU.mult)
                        ht_list[c][j] = ht
                # mm2: chunk-paired, accumulate into yacc; on the last
                # expert stream each finished d-block straight to DRAM
                for d2 in range(DB):
                    ps2c = [psB.tile([128, TC], F32, name=f"ps2_{c}",
                                     tag="psB") for c in range(NCH)]
                    if e == 0:
                        for c in range(NCH):
                            nc.tensor.matmul(
                                ps2c[c][:], b2_sb[:, d2 * 128:(d2 + 1) * 128],
                                combT[:, c * TC:(c + 1) * TC],
                                start=True, stop=False)
                    for j2 in range(JB):
                        for c in range(NCH):
                            nc.tensor.matmul(
                                ps2c[c][:],
                                w2t[j2][:, d2 * 128:(d2 + 1) * 128],
                                ht_list[c][j2][:],
                                start=(e != 0 and j2 == 0),
                                stop=(j2 == JB - 1))
                    for c in range(NCH):
                        ysl = yacc[d2][:, c * TC:(c + 1) * TC]
                        if e == 0:
                            nc.vector.tensor_copy(ysl, ps2c[c][:])
                        else:
                            nc.vector.tensor_tensor(ysl, ysl, ps2c[c][:],
                                                    ALU.add)
                    if e == E - 1:
                        nc.sync.dma_start(out[d2 * 128:(d2 + 1) * 128, :],
                                          yacc[d2][:])

    nc.compile()
    return nc


_PROG = None


def _get_program():
    global _PROG
    if _PROG is None:
        _PROG = _build_program()
    return _PROG


def kernel(x, Wg, bg, W1, b1, W2, b2):
    nc = _get_program()

    x, Wg, bg, W1, b1, W2, b2 = (
        np.asarray(a) for a in (x, Wg, bg, W1, b1, W2, b2))
    xf = np.ascontiguousarray(x.reshape(TOKENS, D).astype(np.float32))
    W1b = np.ascontiguousarray(W1.astype(ml_dtypes.bfloat16))
    W2b = np.ascontiguousarray(W2.astype(ml_dtypes.bfloat16))
    b2b = np.ascontiguousarray(b2.astype(ml_dtypes.bfloat16))
    b1r = np.ascontiguousarray(
        b1.reshape(E, JB, 128).transpose(2, 0, 1).reshape(128, E * JB)
    ).astype(np.float32)
    ones_f = np.ones((1, 128), np.float32)
    eye_f = np.eye(128, dtype=np.float32)
    sel_b = np.zeros((E, E * 128), ml_dtypes.bfloat16)
    for e in range(E):
        sel_b[e, e * 128:(e + 1) * 128] = 1.0

    in_maps = []
    for c in range(N_CORES):
        xt = np.ascontiguousarray(xf[c * T:(c + 1) * T].T)  # [D, T] fp32
        in_maps.append({
            "xt_f": xt,
            "xt_b": xt.astype(ml_dtypes.bfloat16),
            "w1": W1b,
            "w2": W2b,
            "wg": np.ascontiguousarray(Wg.astype(np.float32)),
            "bg": np.ascontiguousarray(bg.astype(np.float32)).reshape(1, E),
            "b1r": b1r,
            "b2": b2b,
            "ones": ones_f,
            "sel": sel_b,
            "eye": eye_f,
        })

    res = bass_utils.run_bass_kernel_spmd(nc, in_maps,
                                          core_ids=list(range(N_CORES)))
    parts = [res.results[c]["out"].T for c in range(N_CORES)]  # [T, D] each
    return np.concatenate(parts, axis=0).reshape(B, S, D).astype(np.float32)


# revision 26
# speedup vs baseline: 1.2993x; 1.0484x over previous
"""MoE (top-2 of 6 experts, dense-expert reference semantics) on 8 TRN2 cores.

Strategy: data-parallel over tokens (8192 tokens -> 1024/core), experts
replicated. Per core:
  - gating in fp32 on the tensor engine (top-2 margins are ~1e-5, bf16 would
    flip selections), top-2 + softmax via vector/scalar engines,
  - per-expert MLP in bf16 (x^T layout, [feature, token]): h^T = W1^T @ x^T,
    gelu(+b1) on ACT, combine-weight fold into h^T on DVE, then the second
    matmul accumulates all experts' contributions plus the combine@b2 term.
  - output written [D, T] per core; host transposes and concatenates.
No collectives needed.
"""

import sys

sys.path.insert(0, "/opt/trn_rl_repo")

import numpy as np
import ml_dtypes

import concourse.bass as bass  # noqa: F401  (registers engine classes)
import concourse.bacc as bacc
import concourse.mybir as mybir
from concourse import tile
from concourse import bass_utils

AF = mybir.ActivationFunctionType
ALU = mybir.AluOpType
AX = mybir.AxisListType
BF16 = mybir.dt.bfloat16
F32 = mybir.dt.float32

N_CORES = 8
B, S, D, E, H = 4, 2048, 1024, 6, 2048
TOKENS = B * S
T = TOKENS // N_CORES  # 1024 tokens per core
TC = 512               # token chunk = matmul moving free dim
NCH = T // TC          # 2 chunks
DB = D // 128          # 8 d blocks
JB = H // 128          # 16 hidden blocks
TB = T // 128          # 8 token blocks (gating)
NEG_BIG = -1.0e30


def _build_program():
    nc = bacc.Bacc("TRN2", target_bir_lowering=False, debug=False,
                   num_devices=N_CORES)

    xt_f = nc.dram_tensor("xt_f", [D, T], F32, kind="ExternalInput").ap()
    w1 = nc.dram_tensor("w1", [E, D, H], BF16, kind="ExternalInput").ap()
    w2 = nc.dram_tensor("w2", [E, H, D], BF16, kind="ExternalInput").ap()
    wg = nc.dram_tensor("wg", [D, E], F32, kind="ExternalInput").ap()
    bgrep = nc.dram_tensor("bgrep", [128, E], F32, kind="ExternalInput").ap()
    b1r = nc.dram_tensor("b1r", [128, E * JB], F32, kind="ExternalInput").ap()
    b2 = nc.dram_tensor("b2", [E, D], BF16, kind="ExternalInput").ap()
    sel = nc.dram_tensor("sel", [E, E * 128], BF16, kind="ExternalInput").ap()
    eye = nc.dram_tensor("eye", [128, 128], F32, kind="ExternalInput").ap()
    out = nc.dram_tensor("out", [D, T], F32, kind="ExternalOutput").ap()

    with tile.TileContext(nc) as tc:
        with (
            tc.tile_pool(name="constp", bufs=1) as constp,
            tc.tile_pool(name="xtfp", bufs=4) as xtfp,
            tc.tile_pool(name="xtbp", bufs=2 * DB) as xtbp,
            tc.tile_pool(name="w1p", bufs=20) as w1p,
            tc.tile_pool(name="w2p", bufs=18) as w2p,
            tc.tile_pool(name="htp", bufs=2 * JB + 2) as htp,
            tc.tile_pool(name="yaccp", bufs=DB) as yaccp,
            tc.tile_pool(name="crepp", bufs=E * NCH) as crepp,
            tc.tile_pool(name="gatp", bufs=4) as gatp,
            tc.tile_pool(name="psA", bufs=4, space="PSUM") as psA,
            tc.tile_pool(name="psB", bufs=4, space="PSUM") as psB,
        ):
            # ---- constants ----
            eye_sb = constp.tile([128, 128], F32, name="eye_sb", tag="eye")
            nc.sync.dma_start(eye_sb[:], eye[:])
            bg_sb = constp.tile([128, E], F32, name="bg_sb", tag="bg")
            nc.sync.dma_start(bg_sb[:], bgrep[:])
            sel_sb = constp.tile([E, E * 128], BF16, name="sel_sb", tag="sel")
            nc.sync.dma_start(sel_sb[:], sel[:])
            b1_sb = constp.tile([128, E * JB], F32, name="b1_sb", tag="b1")
            nc.sync.dma_start(b1_sb[:], b1r[:])
            b2_sb = constp.tile([E, D], BF16, name="b2_sb", tag="b2")
            nc.sync.dma_start(b2_sb[:], b2[:])
            wg_sb = []
            for d in range(DB):
                wgt = constp.tile([128, E], F32, name=f"wg_sb{d}", tag=f"wg{d}")
                nc.sync.dma_start(wgt[:], wg[d * 128:(d + 1) * 128, :])
                wg_sb.append(wgt)
            combT = constp.tile([E, T], BF16, name="combT", tag="combT")

            # ---- priming order: gating x tiles stream first (tiny, lets PE
            # start ~3us in), then expert-0 W1 first half + chunk-0 x^T so
            # mm1 can follow right after gating; the rest stream underneath.
            HH = H // 2
            w1t0 = [[None, None] for _ in range(DB)]
            xtb = [[None, None] for _ in range(DB)]

            # ---- gating (fp32): weight-stationary logits^T, then cheap
            # 6-row transposes back to [token, expert] layout
            logT = constp.tile([E, T], F32, name="logT", tag="logT")
            for c2 in range(NCH):
                ps_l = psB.tile([E, TC], F32, name="ps_l", tag="psB")
                for d in range(DB):
                    xg = xtfp.tile([128, TC], F32, name="xg", tag="xg")
                    nc.sync.dma_start(
                        xg[:], xt_f[d * 128:(d + 1) * 128,
                                    c2 * TC:(c2 + 1) * TC])
                    nc.tensor.matmul(ps_l[:], wg_sb[d][:], xg[:],
                                     start=(d == 0), stop=(d == DB - 1))
                    xb = xtbp.tile([128, TC], BF16, name=f"xtb{d}_{c2}",
                                   tag="xtb")
                    nc.vector.tensor_copy(xb[:], xg[:])
                    xtb[d][c2] = xb
                nc.vector.tensor_copy(logT[:, c2 * TC:(c2 + 1) * TC],
                                      ps_l[:])
            lgs, cmbs = [], []
            for tb in range(TB):
                ps_x = psB.tile([128, E], F32, name="ps_x", tag="psB")
                nc.tensor.transpose(ps_x[:],
                                    logT[:, tb * 128:(tb + 1) * 128],
                                    eye_sb[0:E, 0:E])
                lg = gatp.tile([128, E], F32, name=f"lg{tb}", tag=f"lg{tb}")
                nc.vector.tensor_tensor(lg[:], ps_x[:], bg_sb[:], ALU.add)
                lgs.append(lg)

            # deferred priming: expert-0 weights + x^T (mm1 inputs first)
            for d in range(DB):
                wa = w1p.tile([128, HH], BF16, name=f"w1t0_{d}a", tag="w1")
                nc.sync.dma_start(wa[:], w1[0, d * 128:(d + 1) * 128, 0:HH])
                w1t0[d][0] = wa
            for d in range(DB):
                wb = w1p.tile([128, HH], BF16, name=f"w1t0_{d}b", tag="w1")
                nc.sync.dma_start(wb[:], w1[0, d * 128:(d + 1) * 128, HH:H])
                w1t0[d][1] = wb
            w2t0 = []
            for j in range(JB):
                wt = w2p.tile([128, D], BF16, name=f"w2t0_{j}", tag="w2")
                nc.sync.dma_start(wt[:], w2[0, j * 128:(j + 1) * 128, :])
                w2t0.append(wt)

            def topk_chain(tb):
                lg = lgs[tb]
                m1 = gatp.tile([128, 1], F32, name="m1", tag="m1")
                nc.vector.reduce_max(m1[:], lg[:], axis=AX.X)
                eq1 = gatp.tile([128, E], F32, name="eq1", tag="eq1")
                nc.vector.tensor_scalar(eq1[:], lg[:], m1[:], None,
                                        ALU.is_equal)
                mk = gatp.tile([128, E], F32, name="mk", tag="mk")
                nc.vector.scalar_tensor_tensor(mk[:], eq1[:], NEG_BIG, lg[:],
                                               ALU.mult, ALU.add)
                m2 = gatp.tile([128, 1], F32, name="m2", tag="m2")
                nc.vector.reduce_max(m2[:], mk[:], axis=AX.X)
                eq2 = gatp.tile([128, E], F32, name="eq2", tag="eq2")
                nc.vector.tensor_scalar(eq2[:], mk[:], m2[:], None,
                                        ALU.is_equal)
                dd = gatp.tile([128, 1], F32, name="dd", tag="dd")
                nc.vector.tensor_sub(dd[:], m2[:], m1[:])
                w2s = gatp.tile([128, 1], F32, name="w2s", tag="w2s")
                nc.scalar.activation(w2s[:], dd[:], AF.Sigmoid)
                w1s = gatp.tile([128, 1], F32, name="w1s", tag="w1s")
                nc.vector.tensor_scalar(w1s[:], w2s[:], -1.0, 1.0,
                                        ALU.mult, ALU.add)
                cb1 = gatp.tile([128, E], F32, name="cb1", tag="cb1")
                nc.vector.tensor_scalar(cb1[:], eq1[:], w1s[:], None, ALU.mult)
                cmb = gatp.tile([128, E], F32, name=f"cmb{tb}", tag=f"cmb{tb}")
                nc.vector.scalar_tensor_tensor(cmb[:], eq2[:], w2s[:], cb1[:],
                                               ALU.mult, ALU.add)
                cmbs.append(cmb)

            # process gating tail in chunk-halves so chunk-0 combine
            # weights (and crep tiles) are ready as early as possible
            crep = [[None] * NCH for _ in range(E)]
            TBH = TB // NCH
            for half in range(NCH):
                for tb in range(half * TBH, (half + 1) * TBH):
                    topk_chain(tb)
                for tb in range(half * TBH, (half + 1) * TBH):
                    ps_t = psB.tile([E, 128], F32, name="ps_t", tag="psB")
                    nc.tensor.transpose(ps_t[:], cmbs[tb][:], eye_sb[:])
                    nc.vector.tensor_copy(combT[:, tb * 128:(tb + 1) * 128],
                                          ps_t[:])
                c = half
                for e in range(E):
                    ps_c = psB.tile([128, TC], F32, name="ps_c", tag="psB")
                    nc.tensor.matmul(ps_c[:],
                                     sel_sb[:, e * 128:(e + 1) * 128],
                                     combT[:, c * TC:(c + 1) * TC],
                                     start=True, stop=True)
                    cr = crepp.tile([128, TC], BF16, name=f"crep{e}_{c}",
                                    tag="crep")
                    nc.vector.tensor_copy(cr[:], ps_c[:])
                    crep[e][c] = cr

            # ---- expert loop ----
            yacc = []
            for d in range(DB):
                ya = yaccp.tile([128, T], F32, name=f"yacc{d}", tag="yacc")
                yacc.append(ya)

            for e in range(E):
                if e == 0:
                    w1t, w2t = w1t0, w2t0
                else:
                    w1t = []
                    for d in range(DB):
                        halves = []
                        for h2 in range(2):
                            wt = w1p.tile([128, HH], BF16,
                                          name=f"w1t{e}_{d}{'ab'[h2]}",
                                          tag="w1")
                            nc.sync.dma_start(
                                wt[:], w1[e, d * 128:(d + 1) * 128,
                                          h2 * HH:(h2 + 1) * HH])
                            halves.append(wt)
                        w1t.append(halves)
                    w2t = []
                    for j in range(JB):
                        wt = w2p.tile([128, D], BF16, name=f"w2t{e}_{j}",
                                      tag="w2")
                        nc.sync.dma_start(wt[:],
                                          w2[e, j * 128:(j + 1) * 128, :])
                        w2t.append(wt)

                # mm1: chunk-paired so each stationary W1 slice loads once
                ht_list = [[None] * JB for _ in range(NCH)]
                for j in range(JB):
                    ps1c = [psA.tile([128, TC], F32, name=f"ps1_{c}",
                                     tag="psA") for c in range(NCH)]
                    h2, jj = divmod(j, JB // 2)
                    for d in range(DB):
                        for c in range(NCH):
                            nc.tensor.matmul(
                                ps1c[c][:],
                                w1t[d][h2][:, jj * 128:(jj + 1) * 128],
                                xtb[d][c][:],
                                start=(d == 0), stop=(d == DB - 1))
                    for c in range(NCH):
                        ht = htp.tile([128, TC], BF16, name=f"ht_{c}",
                                      tag="ht")
                        nc.scalar.activation(
                            ht[:], ps1c[c][:], AF.Gelu,
                            bias=b1_sb[:, e * JB + j:e * JB + j + 1])
                        nc.vector.tensor_tensor(ht[:], ht[:], crep[e][c][:],
                                                ALU.mult)
                        ht_list[c][j] = ht
                # mm2: chunk-paired, accumulate into yacc; on the last
                # expert stream each finished d-block straight to DRAM
                for d2 in range(DB):
                    ps2c = [psB.tile([128, TC], F32, name=f"ps2_{c}",
                                     tag="psB") for c in range(NCH)]
                    if e == 0:
                        for c in range(NCH):
                            nc.tensor.matmul(
                                ps2c[c][:], b2_sb[:, d2 * 128:(d2 + 1) * 128],
                                combT[:, c * TC:(c + 1) * TC],
                                start=True, stop=False)
                    for j2 in range(JB):
                        for c in range(NCH):
                            nc.tensor.matmul(
                                ps2c[c][:],
                                w2t[j2][:, d2 * 128:(d2 + 1) * 128],
                                ht_list[c][j2][:],
                                start=(e != 0 and j2 == 0),
                                stop=(j2 == JB - 1))
                    for c in range(NCH):
                        ysl = yacc[d2][:, c * TC:(c + 1) * TC]
                        if e == 0:
                            nc.vector.tensor_copy(ysl, ps2c[c][:])
                        else:
                            nc.vector.tensor_tensor(ysl, ysl, ps2c[c][:],
                                                    ALU.add)
                    if e == E - 1:
                        nc.sync.dma_start(out[d2 * 128:(d2 + 1) * 128, :],
                                          yacc[d2][:])

    nc.compile()
    return nc


_PROG = None


def _get_program():
    global _PROG
    if _PROG is None:
        _PROG = _build_program()
    return _PROG


def kernel(x, Wg, bg, W1, b1, W2, b2):
    nc = _get_program()

    x, Wg, bg, W1, b1, W2, b2 = (
        np.asarray(a) for a in (x, Wg, bg, W1, b1, W2, b2))
    xf = np.ascontiguousarray(x.reshape(TOKENS, D).astype(np.float32))
    W1b = np.ascontiguousarray(W1.astype(ml_dtypes.bfloat16))
    W2b = np.ascontiguousarray(W2.astype(ml_dtypes.bfloat16))
    b2b = np.ascontiguousarray(b2.astype(ml_dtypes.bfloat16))
    b1r = np.ascontiguousarray(
        b1.reshape(E, JB, 128).transpose(2, 0, 1).reshape(128, E * JB)
    ).astype(np.float32)
    bgrep_f = np.ascontiguousarray(
        np.broadcast_to(bg.astype(np.float32).reshape(1, E), (128, E)))
    eye_f = np.eye(128, dtype=np.float32)
    sel_b = np.zeros((E, E * 128), ml_dtypes.bfloat16)
    for e in range(E):
        sel_b[e, e * 128:(e + 1) * 128] = 1.0

    in_maps = []
    for c in range(N_CORES):
        xt = np.ascontiguousarray(xf[c * T:(c + 1) * T].T)  # [D, T] fp32
        in_maps.append({
            "xt_f": xt,
            "w1": W1b,
            "w2": W2b,
            "wg": np.ascontiguousarray(Wg.astype(np.float32)),
            "bgrep": bgrep_f,
            "b1r": b1r,
            "b2": b2b,
            "sel": sel_b,
            "eye": eye_f,
        })

    res = bass_utils.run_bass_kernel_spmd(nc, in_maps,
                                          core_ids=list(range(N_CORES)))
    parts = [res.results[c]["out"].T for c in range(N_CORES)]  # [T, D] each
    return np.concatenate(parts, axis=0).reshape(B, S, D).astype(np.float32)


# revision 29
# speedup vs baseline: 1.3050x; 1.0044x over previous
"""MoE (top-2 of 6 experts, dense-expert reference semantics) on 8 TRN2 cores.

Strategy: data-parallel over tokens (8192 tokens -> 1024/core), experts
replicated. Per core:
  - gating in fp32 on the tensor engine (top-2 margins are ~1e-5, bf16 would
    flip selections), top-2 + softmax via vector/scalar engines,
  - per-expert MLP in bf16 (x^T layout, [feature, token]): h^T = W1^T @ x^T,
    gelu(+b1) on ACT, combine-weight fold into h^T on DVE, then the second
    matmul accumulates all experts' contributions plus the combine@b2 term.
  - output written [D, T] per core; host transposes and concatenates.
No collectives needed.
"""

import sys

sys.path.insert(0, "/opt/trn_rl_repo")

import numpy as np
import ml_dtypes

import concourse.bass as bass  # noqa: F401  (registers engine classes)
import concourse.bacc as bacc
import concourse.mybir as mybir
from concourse import tile
from concourse import bass_utils

AF = mybir.ActivationFunctionType
ALU = mybir.AluOpType
AX = mybir.AxisListType
BF16 = mybir.dt.bfloat16
F32 = mybir.dt.float32

N_CORES = 8
B, S, D, E, H = 4, 2048, 1024, 6, 2048
TOKENS = B * S
T = TOKENS // N_CORES  # 1024 tokens per core
TC = 512               # token chunk = matmul moving free dim
NCH = T // TC          # 2 chunks
DB = D // 128          # 8 d blocks
JB = H // 128          # 16 hidden blocks
TB = T // 128          # 8 token blocks (gating)
NEG_BIG = -1.0e30


def _build_program():
    nc = bacc.Bacc("TRN2", target_bir_lowering=False, debug=False,
                   num_devices=N_CORES)

    xt_f = nc.dram_tensor("xt_f", [D, T], F32, kind="ExternalInput").ap()
    w1 = nc.dram_tensor("w1", [E, D, H], BF16, kind="ExternalInput").ap()
    w2 = nc.dram_tensor("w2", [E, H, D], BF16, kind="ExternalInput").ap()
    wg = nc.dram_tensor("wg", [D, E], F32, kind="ExternalInput").ap()
    bgrep = nc.dram_tensor("bgrep", [128, E], F32, kind="ExternalInput").ap()
    b1r = nc.dram_tensor("b1r", [128, E * JB], F32, kind="ExternalInput").ap()
    b2 = nc.dram_tensor("b2", [E, D], BF16, kind="ExternalInput").ap()
    sel = nc.dram_tensor("sel", [E, E * 128], BF16, kind="ExternalInput").ap()
    eye = nc.dram_tensor("eye", [128, 128], F32, kind="ExternalInput").ap()
    out = nc.dram_tensor("out", [D, T], F32, kind="ExternalOutput").ap()

    with tile.TileContext(nc) as tc:
        with (
            tc.tile_pool(name="constp", bufs=1) as constp,
            tc.tile_pool(name="xtfp", bufs=9) as xtfp,
            tc.tile_pool(name="xtbp", bufs=2 * DB) as xtbp,
            tc.tile_pool(name="w1p", bufs=40) as w1p,
            tc.tile_pool(name="w2p", bufs=18) as w2p,
            tc.tile_pool(name="htp", bufs=2 * JB + 2) as htp,
            tc.tile_pool(name="yaccp", bufs=DB) as yaccp,
            tc.tile_pool(name="crepp", bufs=E * NCH) as crepp,
            tc.tile_pool(name="gatp", bufs=4) as gatp,
            tc.tile_pool(name="psA", bufs=4, space="PSUM") as psA,
            tc.tile_pool(name="psB", bufs=4, space="PSUM") as psB,
        ):
            # ---- constants ----
            eye_sb = constp.tile([128, 128], F32, name="eye_sb", tag="eye")
            nc.sync.dma_start(eye_sb[:], eye[:])
            bg_sb = constp.tile([128, E], F32, name="bg_sb", tag="bg")
            nc.sync.dma_start(bg_sb[:], bgrep[:])
            sel_sb = constp.tile([E, E * 128], BF16, name="sel_sb", tag="sel")
            nc.sync.dma_start(sel_sb[:], sel[:])
            b1_sb = constp.tile([128, E * JB], F32, name="b1_sb", tag="b1")
            nc.sync.dma_start(b1_sb[:], b1r[:])
            b2_sb = constp.tile([E, D], BF16, name="b2_sb", tag="b2")
            nc.sync.dma_start(b2_sb[:], b2[:])
            wg_sb = []
            for d in range(DB):
                wgt = constp.tile([128, E], F32, name=f"wg_sb{d}", tag=f"wg{d}")
                nc.sync.dma_start(wgt[:], wg[d * 128:(d + 1) * 128, :])
                wg_sb.append(wgt)
            combT = constp.tile([E, T], BF16, name="combT", tag="combT")

            # ---- priming order: gating x tiles stream first (tiny, lets PE
            # start ~3us in), then expert-0 W1 first half + chunk-0 x^T so
            # mm1 can follow right after gating; the rest stream underneath.
            HQ = H // 4
            w1t0 = [[None] * 4 for _ in range(DB)]
            xtb = [[None, None] for _ in range(DB)]

            # ---- gating (fp32): weight-stationary logits^T, then cheap
            # 6-row transposes back to [token, expert] layout
            logT = constp.tile([E, T], F32, name="logT", tag="logT")
            xg_tiles = [[None] * DB for _ in range(NCH)]
            for c2 in range(NCH):
                ps_l = psB.tile([E, TC], F32, name="ps_l", tag="psB")
                for d in range(DB):
                    xg = xtfp.tile([128, TC], F32, name="xg", tag="xg")
                    xg_tiles[c2][d] = xg
                    nc.sync.dma_start(
                        xg[:], xt_f[d * 128:(d + 1) * 128,
                                    c2 * TC:(c2 + 1) * TC])
                    nc.tensor.matmul(ps_l[:], wg_sb[d][:], xg[:],
                                     start=(d == 0), stop=(d == DB - 1))
                    xb = xtbp.tile([128, TC], BF16, name=f"xtb{d}_{c2}",
                                   tag="xtb")
                    xtb[d][c2] = xb
                nc.vector.tensor_copy(logT[:, c2 * TC:(c2 + 1) * TC],
                                      ps_l[:])
                for d in range(DB):
                    nc.vector.tensor_copy(xtb[d][c2][:],
                                          xg_tiles[c2][d][:])
            lgs, cmbs = [], []
            for tb in range(TB):
                ps_x = psB.tile([128, E], F32, name="ps_x", tag="psB")
                nc.tensor.transpose(ps_x[:],
                                    logT[:, tb * 128:(tb + 1) * 128],
                                    eye_sb[0:E, 0:E])
                lg = gatp.tile([128, E], F32, name=f"lg{tb}", tag=f"lg{tb}")
                nc.vector.tensor_tensor(lg[:], ps_x[:], bg_sb[:], ALU.add)
                lgs.append(lg)

            # deferred priming: expert-0 W1 in column quarters, first
            # quarter (j0-3) ahead of everything else
            for d in range(DB):
                wa = w1p.tile([128, HQ], BF16, name=f"w1t0_{d}q0", tag="w1")
                nc.sync.dma_start(wa[:], w1[0, d * 128:(d + 1) * 128, 0:HQ])
                w1t0[d][0] = wa
            for q in range(1, 4):
                for d in range(DB):
                    wb = w1p.tile([128, HQ], BF16, name=f"w1t0_{d}q{q}",
                                  tag="w1")
                    nc.sync.dma_start(
                        wb[:], w1[0, d * 128:(d + 1) * 128,
                                  q * HQ:(q + 1) * HQ])
                    w1t0[d][q] = wb
            w2t0 = []
            for j in range(JB):
                wt = w2p.tile([128, D], BF16, name=f"w2t0_{j}", tag="w2")
                nc.sync.dma_start(wt[:], w2[0, j * 128:(j + 1) * 128, :])
                w2t0.append(wt)

            def topk_chain(tb):
                lg = lgs[tb]
                m1 = gatp.tile([128, 1], F32, name="m1", tag="m1")
                nc.vector.reduce_max(m1[:], lg[:], axis=AX.X)
                eq1 = gatp.tile([128, E], F32, name="eq1", tag="eq1")
                nc.vector.tensor_scalar(eq1[:], lg[:], m1[:], None,
                                        ALU.is_equal)
                mk = gatp.tile([128, E], F32, name="mk", tag="mk")
                nc.vector.scalar_tensor_tensor(mk[:], eq1[:], NEG_BIG, lg[:],
                                               ALU.mult, ALU.add)
                m2 = gatp.tile([128, 1], F32, name="m2", tag="m2")
                nc.vector.reduce_max(m2[:], mk[:], axis=AX.X)
                eq2 = gatp.tile([128, E], F32, name="eq2", tag="eq2")
                nc.vector.tensor_scalar(eq2[:], mk[:], m2[:], None,
                                        ALU.is_equal)
                dd = gatp.tile([128, 1], F32, name="dd", tag="dd")
                nc.vector.tensor_sub(dd[:], m2[:], m1[:])
                w2s = gatp.tile([128, 1], F32, name="w2s", tag="w2s")
                nc.scalar.activation(w2s[:], dd[:], AF.Sigmoid)
                w1s = gatp.tile([128, 1], F32, name="w1s", tag="w1s")
                nc.vector.tensor_scalar(w1s[:], w2s[:], -1.0, 1.0,
                                        ALU.mult, ALU.add)
                cb1 = gatp.tile([128, E], F32, name="cb1", tag="cb1")
                nc.vector.tensor_scalar(cb1[:], eq1[:], w1s[:], None, ALU.mult)
                cmb = gatp.tile([128, E], F32, name=f"cmb{tb}", tag=f"cmb{tb}")
                nc.vector.scalar_tensor_tensor(cmb[:], eq2[:], w2s[:], cb1[:],
                                               ALU.mult, ALU.add)
                cmbs.append(cmb)

            # process gating tail in chunk-halves so chunk-0 combine
            # weights (and crep tiles) are ready as early as possible
            crep = [[None] * NCH for _ in range(E)]
            TBH = TB // NCH
            for half in range(NCH):
                for tb in range(half * TBH, (half + 1) * TBH):
                    topk_chain(tb)
                for tb in range(half * TBH, (half + 1) * TBH):
                    ps_t = psB.tile([E, 128], F32, name="ps_t", tag="psB")
                    nc.tensor.transpose(ps_t[:], cmbs[tb][:], eye_sb[:])
                    nc.vector.tensor_copy(combT[:, tb * 128:(tb + 1) * 128],
                                          ps_t[:])
                c = half
                for e in range(E):
                    ps_c = psB.tile([128, TC], F32, name="ps_c", tag="psB")
                    nc.tensor.matmul(ps_c[:],
                                     sel_sb[:, e * 128:(e + 1) * 128],
                                     combT[:, c * TC:(c + 1) * TC],
                                     start=True, stop=True)
                    cr = crepp.tile([128, TC], BF16, name=f"crep{e}_{c}",
                                    tag="crep")
                    nc.vector.tensor_copy(cr[:], ps_c[:])
                    crep[e][c] = cr

            # ---- expert loop ----
            yacc = []
            for d in range(DB):
                ya = yaccp.tile([128, T], F32, name=f"yacc{d}", tag="yacc")
                yacc.append(ya)

            for e in range(E):
                if e == 0:
                    w1t, w2t = w1t0, w2t0
                else:
                    w1t = []
                    for d in range(DB):
                        quarters = []
                        for q in range(4):
                            wt = w1p.tile([128, HQ], BF16,
                                          name=f"w1t{e}_{d}q{q}",
                                          tag="w1")
                            nc.sync.dma_start(
                                wt[:], w1[e, d * 128:(d + 1) * 128,
                                          q * HQ:(q + 1) * HQ])
                            quarters.append(wt)
                        w1t.append(quarters)
                    w2t = []
                    for j in range(JB):
                        wt = w2p.tile([128, D], BF16, name=f"w2t{e}_{j}",
                                      tag="w2")
                        nc.sync.dma_start(wt[:],
                                          w2[e, j * 128:(j + 1) * 128, :])
                        w2t.append(wt)

                # mm1: chunk-paired so each stationary W1 slice loads once
                ht_list = [[None] * JB for _ in range(NCH)]
                for j in range(JB):
                    ps1c = [psA.tile([128, TC], F32, name=f"ps1_{c}",
                                     tag="psA") for c in range(NCH)]
                    q, jj = divmod(j, 4)
                    for d in range(DB):
                        for c in range(NCH):
                            nc.tensor.matmul(
                                ps1c[c][:],
                                w1t[d][q][:, jj * 128:(jj + 1) * 128],
                                xtb[d][c][:],
                                start=(d == 0), stop=(d == DB - 1))
                    for c in range(NCH):
                        ht = htp.tile([128, TC], BF16, name=f"ht_{c}",
                                      tag="ht")
                        nc.scalar.activation(
                            ht[:], ps1c[c][:], AF.Gelu,
                            bias=b1_sb[:, e * JB + j:e * JB + j + 1])
                        nc.vector.tensor_tensor(ht[:], ht[:], crep[e][c][:],
                                                ALU.mult)
                        ht_list[c][j] = ht
                # mm2: chunk-paired, accumulate into yacc; on the last
                # expert stream each finished d-block straight to DRAM
                for d2 in range(DB):
                    ps2c = [psB.tile([128, TC], F32, name=f"ps2_{c}",
                                     tag="psB") for c in range(NCH)]
                    if e == 0:
                        for c in range(NCH):
                            nc.tensor.matmul(
                                ps2c[c][:], b2_sb[:, d2 * 128:(d2 + 1) * 128],
                                combT[:, c * TC:(c + 1) * TC],
                                start=True, stop=False)
                    for j2 in range(JB):
                        for c in range(NCH):
                            nc.tensor.matmul(
                                ps2c[c][:],
                                w2t[j2][:, d2 * 128:(d2 + 1) * 128],
                                ht_list[c][j2][:],
                                start=(e != 0 and j2 == 0),
                                stop=(j2 == JB - 1))
                    for c in range(NCH):
                        ysl = yacc[d2][:, c * TC:(c + 1) * TC]
                        if e == 0:
                            nc.vector.tensor_copy(ysl, ps2c[c][:])
                        else:
                            nc.vector.tensor_tensor(ysl, ysl, ps2c[c][:],
                                                    ALU.add)
                    if e == E - 1:
                        nc.sync.dma_start(out[d2 * 128:(d2 + 1) * 128, :],
                                          yacc[d2][:])

    nc.compile()
    return nc


_PROG = None


def _get_program():
    global _PROG
    if _PROG is None:
        _PROG = _build_program()
    return _PROG


def kernel(x, Wg, bg, W1, b1, W2, b2):
    nc = _get_program()

    x, Wg, bg, W1, b1, W2, b2 = (
        np.asarray(a) for a in (x, Wg, bg, W1, b1, W2, b2))
    xf = np.ascontiguousarray(x.reshape(TOKENS, D).astype(np.float32))
    W1b = np.ascontiguousarray(W1.astype(ml_dtypes.bfloat16))
    W2b = np.ascontiguousarray(W2.astype(ml_dtypes.bfloat16))
    b2b = np.ascontiguousarray(b2.astype(ml_dtypes.bfloat16))
    b1r = np.ascontiguousarray(
        b1.reshape(E, JB, 128).transpose(2, 0, 1).reshape(128, E * JB)
    ).astype(np.float32)
    bgrep_f = np.ascontiguousarray(
        np.broadcast_to(bg.astype(np.float32).reshape(1, E), (128, E)))
    eye_f = np.eye(128, dtype=np.float32)
    sel_b = np.zeros((E, E * 128), ml_dtypes.bfloat16)
    for e in range(E):
        sel_b[e, e * 128:(e + 1) * 128] = 1.0

    in_maps = []
    for c in range(N_CORES):
        xt = np.ascontiguousarray(xf[c * T:(c + 1) * T].T)  # [D, T] fp32
        in_maps.append({
            "xt_f": xt,
            "w1": W1b,
            "w2": W2b,
            "wg": np.ascontiguousarray(Wg.astype(np.float32)),
            "bgrep": bgrep_f,
            "b1r": b1r,
            "b2": b2b,
            "sel": sel_b,
            "eye": eye_f,
        })

    res = bass_utils.run_bass_kernel_spmd(nc, in_maps,
                                          core_ids=list(range(N_CORES)))
    parts = [res.results[c]["out"].T for c in range(N_CORES)]  # [T, D] each
    return np.concatenate(parts, axis=0).reshape(B, S, D).astype(np.float32)
